# revision 13
# baseline (speedup 1.0000x reference)
"""Trainium2 Bass kernel for nn_DGAD_net (vq_codebook).

Strategy
--------
The reference's dominant cost is ``einsum('bchw,oc->bohw', inter, w).mean((2,3))``
followed by tiny MLPs.  The einsum and the spatial mean commute (both linear),
so on device we only *sum-pool* the feature maps and fold ``w / (H*W)`` into the
first matmul — the problem becomes DMA/reduction bound instead of 100+ GFLOP of
conv.

Data parallel over 8 NeuronCores: batch 512 -> 64 per core.  Weights are
host-preprocessed (transposed / permuted / scaled / concat-split) and
replicated.  Each core returns a [1, 264] row of per-sample partials:
  [osvdd_main(64) | csvdd_main(64) | osvdd_aug(64) | csvdd_aug(64)
   | sum(q0*ls1)(4) | sum(q1*ls0)(4)]
and the host reduces those to the [4, 1] output.

Trace-driven tuning (v3, 357us vs 412us baseline):
 * Big tiles — 4-sample inter (3.2 MB) / 16-sample origin (1.6 MB) —
   alternating across the SP+ACT HWDGE rings: sustains ~418 GB/s
   through the whole stream (the old 400 KB origin DMAs were
   issue-bound at ~50% bandwidth).
 * Load order xi -> ai -> xo -> ao with phase-interleaved emission:
   branch-0's sfc chain is emitted mid-ai so its DVE ops never block
   the streaming pool reduces; the kernel tail is branch-1's MLP chains
   (PE-bound, ~60us of LDWEIGHTS+MATMUL pairs).
 * DVE reduce is PERF_ONE (~1.04 cy/elem regardless of dtype); a
   TensorTensor pre-fold halves the reduce itself but TT is also
   1 elem/lane/cycle here, so DVE stays ~118 Ge/s ≈ 262us total —
   just under the 279us DMA floor.
Dead ends (measured): SWDGE f32->bf16 cast-DMA runs at only ~150-200
GB/s and bf16 does NOT speed up the PERF_ONE reduce (v2, 404us);
weights or late origin tiles on the gpsimd ring fire at t~0 (its engine
queue is empty, so issue order decouples from emission order) and
disrupt the stream (v4 410us / v5 420us).

The distill term only needs softmax/log_softmax of ``sim`` which are invariant
to per-row shifts, so the device computes ``score[b,k] = 2 t.p_k - ||p_k||^2``
(skipping ``-||t||^2``) — same softmax, same argmax.

This walrus build only encodes ONE sync wait per instruction; ``_split_waits``
rewrites the traced BIR, moving excess waits onto preceding same-engine NOPs.
"""

import sys

for _p in ("/opt/trn_rl_repo", "/root/.axon_site/_ro/trn_rl_repo"):
    if _p not in sys.path:
        sys.path.append(_p)

import numpy as np

B, CI, HW2, CO, HO2, D, DOM = 512, 256, 784, 512, 49, 64, 4
NCORE = 8
BC = B // NCORE  # 64 samples per core
EPOCHS = 30
W_TEMP, T_TEMP = 0.7, 0.4
_SCHED = np.concatenate(
    [np.linspace(W_TEMP, T_TEMP, int(EPOCHS * 0.25)),
     np.ones(EPOCHS - int(EPOCHS * 0.25)) * T_TEMP]
)

_NC = None  # built once per process
_ctr = [0]


def _split_waits(nc, mybir, cap=1):
    """Move excess sync waits onto preceding same-engine NOPs (this
    walrus encodes at most `cap` waits per instruction).  Same-engine
    program order preserves semantics exactly."""
    for f in nc.m.functions:
        for bb in f.blocks:
            new = []
            for inst in bb.instructions:
                si = inst.sync_info
                if si is not None and si.on_wait and len(si.on_wait) > cap:
                    waits = list(si.on_wait)
                    excess, keep = waits[:-cap], waits[-cap:]
                    while excess:
                        chunk, excess = excess[:cap], excess[cap:]
                        nop = mybir.InstNoOp(
                            name=f"I-wsplit-{_ctr[0]}", ins=[], outs=[]
                        )
                        _ctr[0] += 1
                        nop.engine = inst.engine
                        nop.sync_info = mybir.SyncInfo(on_wait=chunk, on_update=[])
                        new.append(nop)
                    inst.sync_info = mybir.SyncInfo(
                        on_wait=keep, on_update=list(si.on_update)
                    )
                new.append(inst)
            bb.instructions = new


def _build_nc():
    import concourse.bass as bass
    import concourse.tile as tile
    from concourse import mybir
    from contextlib import ExitStack

    AF = mybir.ActivationFunctionType
    AL = mybir.AluOpType
    AX = mybir.AxisListType
    f32 = mybir.dt.float32
    bf16 = mybir.dt.bfloat16

    nc = bass.Bass(trn_type="TRN2")

    # ---- DRAM I/O ----
    xi = nc.dram_tensor("xi", [BC, CI * HW2], f32, kind="ExternalInput")
    xo = nc.dram_tensor("xo", [BC, CO * HO2], f32, kind="ExternalInput")
    ai = nc.dram_tensor("ai", [BC, CI * HW2], f32, kind="ExternalInput")
    ao = nc.dram_tensor("ao", [BC, CO * HO2], f32, kind="ExternalInput")
    w_sw = nc.dram_tensor("w_sw", [2, 128, 512], bf16, kind="ExternalInput")
    w_s1 = nc.dram_tensor("w_s1", [4, 128, 1024], bf16, kind="ExternalInput")
    w_s2 = nc.dram_tensor("w_s2", [8, 128, 512], bf16, kind="ExternalInput")
    w_s3 = nc.dram_tensor("w_s3", [4, 128, 64], bf16, kind="ExternalInput")
    w_o1 = nc.dram_tensor("w_o1", [4, 128, 1024], bf16, kind="ExternalInput")
    w_o2 = nc.dram_tensor("w_o2", [8, 128, 512], bf16, kind="ExternalInput")
    w_o3 = nc.dram_tensor("w_o3", [4, 128, 64], bf16, kind="ExternalInput")
    w_t1 = nc.dram_tensor("w_t1", [64, 64], bf16, kind="ExternalInput")
    w_t2 = nc.dram_tensor("w_t2", [64, 64], bf16, kind="ExternalInput")
    w_c1 = nc.dram_tensor("w_c1", [64, 64], bf16, kind="ExternalInput")
    w_c2 = nc.dram_tensor("w_c2", [64, 64], bf16, kind="ExternalInput")
    w_q1 = nc.dram_tensor("w_q1", [64, 64], bf16, kind="ExternalInput")
    w_q2 = nc.dram_tensor("w_q2", [64, 64], bf16, kind="ExternalInput")
    w_pc = nc.dram_tensor("w_pc", [4, 64], bf16, kind="ExternalInput")
    w_p2 = nc.dram_tensor("w_p2", [65, 4], f32, kind="ExternalInput")
    b_t1 = nc.dram_tensor("b_t1", [64, 1], f32, kind="ExternalInput")
    b_cn = nc.dram_tensor("b_cn", [64, 1], f32, kind="ExternalInput")
    invt = nc.dram_tensor("invt", [64, 1], f32, kind="ExternalInput")
    idm = nc.dram_tensor("idm", [64, 64], f32, kind="ExternalInput")
    out = nc.dram_tensor("out", [1, 264], f32, kind="ExternalOutput")

    with tile.TileContext(nc) as tc:
        with ExitStack() as ctx:
            wp = ctx.enter_context(tc.tile_pool(name="wp", bufs=1))
            ip = ctx.enter_context(tc.tile_pool(name="ip", bufs=4))
            op = ctx.enter_context(tc.tile_pool(name="op", bufs=4))
            plp = ctx.enter_context(tc.tile_pool(name="plp", bufs=1))
            ap = ctx.enter_context(tc.tile_pool(name="ap", bufs=1))
            sp = ctx.enter_context(tc.tile_pool(name="sp", bufs=1))
            psA = ctx.enter_context(tc.tile_pool(name="psA", bufs=4, space="PSUM"))
            psB = ctx.enter_context(tc.tile_pool(name="psB", bufs=1, space="PSUM"))

            def wtile(h, shape, tag, rearr=None, dt=f32):
                t = wp.tile(shape, dt, tag=tag)
                src = h[:]
                if rearr:
                    src = src.rearrange(rearr)
                nc.scalar.dma_start(t[:], src)
                return t

            sw_sb = wtile(w_sw, [128, 2, 512], "w_sw", "m p j -> p m j", bf16)
            s1_sb = wtile(w_s1, [128, 4, 1024], "w_s1", "m p j -> p m j", bf16)
            s2_sb = wtile(w_s2, [128, 8, 512], "w_s2", "m p j -> p m j", bf16)
            s3_sb = wtile(w_s3, [128, 4, 64], "w_s3", "m p j -> p m j", bf16)
            o1_sb = wtile(w_o1, [128, 4, 1024], "w_o1", "m p j -> p m j", bf16)
            o2_sb = wtile(w_o2, [128, 8, 512], "w_o2", "m p j -> p m j", bf16)
            o3_sb = wtile(w_o3, [128, 4, 64], "w_o3", "m p j -> p m j", bf16)
            t1_sb = wtile(w_t1, [64, 64], "w_t1", dt=bf16)
            t2_sb = wtile(w_t2, [64, 64], "w_t2", dt=bf16)
            c1_sb = wtile(w_c1, [64, 64], "w_c1", dt=bf16)
            c2_sb = wtile(w_c2, [64, 64], "w_c2", dt=bf16)
            q1_sb = wtile(w_q1, [64, 64], "w_q1", dt=bf16)
            q2_sb = wtile(w_q2, [64, 64], "w_q2", dt=bf16)
            pc_sb = wtile(w_pc, [4, 64], "w_pc", dt=bf16)
            p2_sb = wtile(w_p2, [65, 4], "w_p2")
            bt1_sb = wtile(b_t1, [64, 1], "b_t1")
            bcn_sb = wtile(b_cn, [64, 1], "b_cn")
            it_sb = wtile(invt, [64, 1], "invt")
            id_sb = wtile(idm, [64, 64], "idm")

            ones_sb = wp.tile([64, 1], f32, tag="ones")
            nc.vector.memset(ones_sb[:], 1.0)
            out_sb = wp.tile([1, 264], f32, tag="out_sb")

            # ---- pooling ----
            # 4-sample inter tiles (3.2 MB) / 16-sample origin tiles
            # (1.6 MB), alternating SP / ACT HWDGE rings (two rings
            # together sustain ~415 GB/s; small tiles were issue-bound).
            # A TensorTensor pre-fold (PERF_TWO datapath, 2 out/lane/cy)
            # halves the element count before the PERF_ONE reduce.
            def pool_inter_phase(xh, dst, t0, t1):
                """tiles t0..t1-1 of 4 samples each.  dst: [128,2,BC] f32."""
                for ti in range(t0, t1):
                    b0 = ti * 4
                    t = ip.tile([128, 4, 2 * HW2], f32, tag="it")
                    eng = nc.sync if ti % 2 == 0 else nc.scalar
                    eng.dma_start(
                        t[:], xh[b0:b0 + 4].rearrange("b (p f) -> p b f", p=128)
                    )
                    a = t[:].rearrange("p b (g h) -> p b g h", g=2)
                    nc.vector.tensor_add(
                        a[:, :, :, 0:392], a[:, :, :, 0:392], a[:, :, :, 392:784]
                    )
                    nc.vector.reduce_sum(
                        dst[:, :, b0:b0 + 4].rearrange("p g b -> p b g"),
                        a[:, :, :, 0:392],
                        axis=AX.X,
                    )

            def pool_origin_phase(xh, dst):
                """dst: [128, 4, BC] f32.  4 tiles of 16 samples."""
                for ti in range(4):
                    b0 = ti * 16
                    t = op.tile([128, 16, 4 * HO2], f32, tag="ot")
                    eng = nc.sync if ti % 2 == 0 else nc.scalar
                    eng.dma_start(
                        t[:], xh[b0:b0 + 16].rearrange("b (p f) -> p b f", p=128)
                    )
                    nc.vector.reduce_sum(
                        dst[:, :, b0:b0 + 16].rearrange("p g b -> p b g"),
                        t[:].rearrange("p b (g h) -> p b g h", g=4),
                        axis=AX.X,
                    )

            def cast_pool(dst, tag, g):
                db = plp.tile([128, g, BC], bf16, tag=tag)
                nc.vector.tensor_copy(db[:], dst[:])
                return db

            # ---- MLP helpers ----
            def chunk_layer(w_sb, ins, nout_chunks, outw, r, nm, act=True):
                """outT chunks [outw, BC] = Lrelu( sum_m w_sb[:, m, chunk] @ ins[m] )."""
                outs = []
                nin = len(ins)
                for m2 in range(nout_chunks):
                    ps = psA.tile([128, BC], f32, tag="mm")
                    for m in range(nin):
                        nc.tensor.matmul(
                            ps[:outw, :],
                            w_sb[:, m, m2 * outw:(m2 + 1) * outw],
                            ins[m][:],
                            start=(m == 0),
                            stop=(m == nin - 1),
                        )
                    tl = ap.tile([outw, BC], bf16, tag=f"r{r}{nm}{m2}")
                    if act:
                        nc.scalar.activation(tl[:], ps[:outw, :], AF.Lrelu,
                                             alpha=0.01)
                    else:
                        nc.scalar.copy(tl[:], ps[:outw, :])
                    outs.append(tl)
                return outs

            def small_mm(lhsT, rhs, r, nm, act=True, bias=None, extra=None):
                """[64, BC] = act(lhsT.T @ rhs [+ extra matmul] + bias)."""
                ps = psA.tile([128, BC], f32, tag="mm")
                nc.tensor.matmul(
                    ps[:64, :], lhsT[:], rhs[:],
                    start=True, stop=(extra is None),
                )
                if extra is not None:
                    nc.tensor.matmul(
                        ps[:64, :], extra[0][:], extra[1][:], start=False, stop=True
                    )
                tl = ap.tile([64, BC], bf16, tag=f"r{r}{nm}")
                if bias is not None:
                    nc.scalar.activation(
                        tl[:], ps[:64, :], AF.Lrelu, bias=bias[:], alpha=0.01
                    )
                else:
                    nc.scalar.activation(tl[:], ps[:64, :], AF.Lrelu, alpha=0.01)
                return tl

            def svdd_row(featT, r, nm, off):
                """out_sb[0, off:off+64] = || featT[:, b] - center ||^2 per b.
                Square(x + (-c)) in ONE ACT op: keeps the ofc chains
                entirely off DVE so late origin-pool reduces are never
                queued behind them."""
                sq = sp.tile([64, BC], f32, tag=f"r{r}{nm}sq")
                nc.scalar.activation(sq[:], featT[:], AF.Square, bias=bcn_sb[:])
                pr = psB.tile([1, BC], f32, tag="row")
                nc.tensor.matmul(pr[:], ones_sb[:], sq[:])
                nc.scalar.copy(out_sb[:, off:off + BC], pr[:])

            # distill helpers
            def teacher_q(score, mx, r):
                nb = sp.tile([64, 1], f32, tag=f"nb{r}")
                nc.vector.tensor_scalar(nb[:], mx[:], it_sb[:], -1.0,
                                        op0=AL.mult, op1=AL.mult)
                e = sp.tile([64, 4], f32, tag=f"te{r}")
                es = sp.tile([64, 1], f32, tag=f"tes{r}")
                nc.scalar.activation(e[:], score[:], AF.Exp, bias=nb[:],
                                     scale=it_sb[:], accum_out=es[:])
                rc = sp.tile([64, 1], f32, tag=f"trc{r}")
                nc.vector.reciprocal(rc[:], es[:])
                q = sp.tile([64, 4], f32, tag=f"tq{r}")
                nc.vector.tensor_scalar(q[:], e[:], rc[:], None, op0=AL.mult)
                return q

            def student_ls(score, mx, r):
                nb = sp.tile([64, 1], f32, tag=f"snb{r}")
                nc.vector.tensor_scalar(nb[:], mx[:], -1.0, None, op0=AL.mult)
                e = sp.tile([64, 4], f32, tag=f"se{r}")
                es = sp.tile([64, 1], f32, tag=f"ses{r}")
                nc.scalar.activation(e[:], score[:], AF.Exp, bias=nb[:],
                                     accum_out=es[:])
                ln = sp.tile([64, 1], f32, tag=f"sln{r}")
                nc.scalar.activation(ln[:], es[:], AF.Ln)
                lse = sp.tile([64, 1], f32, tag=f"slse{r}")
                nc.vector.tensor_add(lse[:], ln[:], mx[:])
                ls = sp.tile([64, 4], f32, tag=f"sls{r}")
                nc.vector.tensor_scalar(ls[:], score[:], lse[:], None,
                                        op0=AL.subtract)
                return ls

            scores, qs, lss, ohTs = {}, {}, {}, {}

            def sfc_part(r, pib):
                """inter-dependent chain: shallow conv + sfc + texture +
                score + distill stats + onehot prototype pick."""
                sh = chunk_layer(sw_sb, [pib[:, g, :] for g in range(2)], 4,
                                 128, r, "sh", act=False)
                a1 = chunk_layer(s1_sb, sh, 8, 128, r, "a1")
                a2 = chunk_layer(s2_sb, a1, 4, 128, r, "a2")
                ps = psA.tile([128, BC], f32, tag="mm")
                for m in range(4):
                    nc.tensor.matmul(ps[:64, :], s3_sb[:, m, :], a2[m][:],
                                     start=(m == 0), stop=(m == 3))
                sT = ap.tile([64, BC], bf16, tag=f"r{r}sT")
                nc.scalar.activation(sT[:], ps[:64, :], AF.Lrelu, alpha=0.01)

                t1 = small_mm(t1_sb, sT, r, "t1", bias=bt1_sb)
                ps = psA.tile([128, BC], f32, tag="mm")
                nc.tensor.matmul(ps[:64, :], t2_sb[:], t1[:])
                tx = ap.tile([65, BC], f32, tag=f"r{r}tx")
                nc.scalar.activation(tx[0:64, :], ps[:64, :], AF.Lrelu, alpha=0.01)
                nc.vector.memset(tx[64:65, :], 1.0)

                pss = psB.tile([64, 4], f32, tag="sc")
                nc.tensor.matmul(pss[:], tx[:], p2_sb[:])
                score = sp.tile([64, 4], f32, tag=f"score{r}")
                nc.scalar.copy(score[:], pss[:])
                mx = sp.tile([64, 1], f32, tag=f"mx{r}")
                nc.vector.reduce_max(mx[:], score[:], axis=AX.X)
                scores[r] = (score, mx)
                qs[r] = teacher_q(score, mx, r)
                lss[r] = student_ls(score, mx, r)

                oh1 = sp.tile([64, 4], f32, tag=f"oh{r}")
                nc.vector.tensor_scalar(oh1[:], score[:], mx[:], None,
                                        op0=AL.is_ge)
                psT = psB.tile([4, 64], f32, tag="ohT")
                nc.tensor.transpose(psT[:], oh1[:], id_sb[:])
                ohT = sp.tile([4, 64], bf16, tag=f"ohT{r}")
                nc.scalar.copy(ohT[:], psT[:])
                ohTs[r] = ohT

            def ofc_part(r, pob):
                """origin-dependent chain: ofc + cfc + oc head + svdd rows."""
                b1 = chunk_layer(o1_sb, [pob[:, g, :] for g in range(4)], 8,
                                 128, r, "b1")
                b2 = chunk_layer(o2_sb, b1, 4, 128, r, "b2")
                ps = psA.tile([128, BC], f32, tag="mm")
                for m in range(4):
                    nc.tensor.matmul(ps[:64, :], o3_sb[:, m, :], b2[m][:],
                                     start=(m == 0), stop=(m == 3))
                orT = ap.tile([64, BC], bf16, tag=f"r{r}orT")
                nc.scalar.activation(orT[:], ps[:64, :], AF.Lrelu, alpha=0.01)

                cf1 = small_mm(c1_sb, orT, r, "cf1", extra=(pc_sb, ohTs[r]))
                clsT = small_mm(c2_sb, cf1, r, "cls")
                svdd_row(clsT, r, "c", off=(64 if r == 0 else 192))

                g1 = small_mm(q1_sb, orT, r, "g1")
                g2 = small_mm(q2_sb, g1, r, "g2")
                svdd_row(g2, r, "o", off=(0 if r == 0 else 128))

            # ================= emission schedule =================
            pi0 = plp.tile([128, 2, BC], f32, tag="pi0")
            pi1 = plp.tile([128, 2, BC], f32, tag="pi1")
            po0 = plp.tile([128, 4, BC], f32, tag="po0")
            po1 = plp.tile([128, 4, BC], f32, tag="po1")

            # xi
            pool_inter_phase(xi, pi0, 0, 16)
            pi0b = cast_pool(pi0, "pi0b", 2)
            # ai first half
            pool_inter_phase(ai, pi1, 0, 8)
            # branch-0 inter chain (its DVE ops land behind ai[0:8] reduces)
            sfc_part(0, pi0b)
            # ai second half
            pool_inter_phase(ai, pi1, 8, 16)
            pi1b = cast_pool(pi1, "pi1b", 2)
            # all origin DMA phases BEFORE the late compute chains: svdd is
            # ACT-only now, so neither the ACT queue (DMA issues) nor the
            # DVE queue (pool reduces) ever waits behind compute
            pool_origin_phase(xo, po0)
            po0b = cast_pool(po0, "po0b", 4)
            pool_origin_phase(ao, po1)
            po1b = cast_pool(po1, "po1b", 4)
            # branch-1 inter chain, then both origin chains
            sfc_part(1, pi1b)
            ofc_part(0, po0b)
            ofc_part(1, po1b)

            # distill cross terms
            pr01 = sp.tile([64, 4], f32, tag="pr01")
            nc.vector.tensor_mul(pr01[:], qs[0][:], lss[1][:])
            pc01 = psB.tile([1, 4], f32, tag="pc")
            nc.tensor.matmul(pc01[:], ones_sb[:], pr01[:])
            nc.scalar.copy(out_sb[:, 256:260], pc01[:])

            pr10 = sp.tile([64, 4], f32, tag="pr10")
            nc.vector.tensor_mul(pr10[:], qs[1][:], lss[0][:])
            pc10 = psB.tile([1, 4], f32, tag="pc")
            nc.tensor.matmul(pc10[:], ones_sb[:], pr10[:])
            nc.scalar.copy(out_sb[:, 260:264], pc10[:])

            nc.sync.dma_start(out[:], out_sb[:])

    _split_waits(nc, mybir)
    return nc


def _get_nc():
    global _NC
    if _NC is None:
        _NC = _build_nc()
    return _NC


def _prep_weights(shallow_conv_w, ofc_w1, ofc_w2, ofc_w3, sfc_w1, sfc_w2, sfc_w3,
                  tfc_w1, tfc_w2, cfc_w1, cfc_w2, oc_w1, oc_w2, center, protos,
                  epoch):
    f = np.float32
    sw = np.asarray(shallow_conv_w, f)
    o1, o2, o3 = (np.asarray(a, f) for a in (ofc_w1, ofc_w2, ofc_w3))
    s1, s2, s3 = (np.asarray(a, f) for a in (sfc_w1, sfc_w2, sfc_w3))
    t1, t2 = np.asarray(tfc_w1, f), np.asarray(tfc_w2, f)
    c1, c2 = np.asarray(cfc_w1, f), np.asarray(cfc_w2, f)
    q1, q2 = np.asarray(oc_w1, f), np.asarray(oc_w2, f)
    ctr = np.asarray(center, f)
    pr = np.asarray(protos, f)

    import ml_dtypes
    bf = ml_dtypes.bfloat16

    w = {}
    # channel c = 2p+g for inter (1568 floats per partition), 4p+g for origin
    w["w_sw"] = np.ascontiguousarray(
        (sw.T / HW2).astype(f).reshape(128, 2, 512).transpose(1, 0, 2)).astype(bf)
    w["w_s1"] = np.ascontiguousarray(s1.T.reshape(4, 128, 1024)).astype(bf)
    w["w_s2"] = np.ascontiguousarray(s2.T.reshape(8, 128, 512)).astype(bf)
    w["w_s3"] = np.ascontiguousarray(s3.T.reshape(4, 128, 64)).astype(bf)
    w["w_o1"] = np.ascontiguousarray(
        (o1.T / HO2).astype(f).reshape(128, 4, 1024).transpose(1, 0, 2)).astype(bf)
    w["w_o2"] = np.ascontiguousarray(o2.T.reshape(8, 128, 512)).astype(bf)
    w["w_o3"] = np.ascontiguousarray(o3.T.reshape(4, 128, 64)).astype(bf)
    ta, tb = t1[:, :64], t1[:, 64:]
    w["w_t1"] = np.ascontiguousarray((ta + tb).T).astype(bf)
    w["b_t1"] = np.ascontiguousarray(-(tb @ ctr))[:, None]
    w["w_t2"] = np.ascontiguousarray(t2.T).astype(bf)
    ca, cb = c1[:, :64], c1[:, 64:]
    w["w_c1"] = np.ascontiguousarray((ca + cb).T).astype(bf)
    w["w_pc"] = np.ascontiguousarray(-(pr @ cb.T)).astype(bf)
    w["w_c2"] = np.ascontiguousarray(c2.T).astype(bf)
    w["w_q1"] = np.ascontiguousarray(q1.T).astype(bf)
    w["w_q2"] = np.ascontiguousarray(q2.T).astype(bf)
    p2 = np.concatenate([2.0 * pr.T, -(pr ** 2).sum(1)[None, :]], 0).astype(f)
    w["w_p2"] = np.ascontiguousarray(p2)
    w["b_cn"] = np.ascontiguousarray(-ctr)[:, None]
    temp = f(_SCHED[int(np.asarray(epoch))])
    w["invt"] = np.full((64, 1), 1.0 / temp, f)
    w["idm"] = np.eye(64, dtype=f)
    return w


def _run(inputs, trace=False):
    from concourse.bass_utils import run_bass_kernel_spmd

    nc = _get_nc()
    f = np.float32
    inter = np.asarray(inputs["inter_feat"], f).reshape(B, CI * HW2)
    orig = np.asarray(inputs["origin_feat"], f).reshape(B, CO * HO2)
    ainter = np.asarray(inputs["aug_inter_feat"], f).reshape(B, CI * HW2)
    aorig = np.asarray(inputs["aug_origin_feat"], f).reshape(B, CO * HO2)
    w = _prep_weights(
        inputs["shallow_conv_w"], inputs["ofc_w1"], inputs["ofc_w2"],
        inputs["ofc_w3"], inputs["sfc_w1"], inputs["sfc_w2"], inputs["sfc_w3"],
        inputs["tfc_w1"], inputs["tfc_w2"], inputs["cfc_w1"], inputs["cfc_w2"],
        inputs["oc_w1"], inputs["oc_w2"], inputs["center"], inputs["protos"],
        inputs["epoch"],
    )
    in_maps = []
    for c in range(NCORE):
        sl = slice(c * BC, (c + 1) * BC)
        m = dict(w)
        m["xi"] = np.ascontiguousarray(inter[sl])
        m["xo"] = np.ascontiguousarray(orig[sl])
        m["ai"] = np.ascontiguousarray(ainter[sl])
        m["ao"] = np.ascontiguousarray(aorig[sl])
        in_maps.append(m)

    res = run_bass_kernel_spmd(nc, in_maps, core_ids=list(range(NCORE)),
                               trace=trace)
    rows = np.stack([res.results[c]["out"][0] for c in range(NCORE)])  # [8, 264]
    osv0 = rows[:, 0:64].astype(f)
    csv0 = rows[:, 64:128].astype(f)
    osv1 = rows[:, 128:192].astype(f)
    csv1 = rows[:, 192:256].astype(f)
    s01 = rows[:, 256:260].astype(f)
    s10 = rows[:, 260:264].astype(f)

    l01 = f(-(s01.sum(dtype=f)) / B)
    l10 = f(-(s10.sum(dtype=f)) / B)
    distill = f((l01 + l10) / 2.0)
    row_o = f(osv0.sum(dtype=f) / B + osv1.sum(dtype=f) / B)
    row_c = f(csv0.sum(dtype=f) / B + csv1.sum(dtype=f) / B)
    row_a = f(np.abs(osv0 - csv0).sum(dtype=f) / B
              + np.abs(osv1 - csv1).sum(dtype=f) / B)
    out = np.array([[distill], [row_o], [row_c], [row_a]], dtype=f)
    return out, res


def kernel(**inputs):
    out, _ = _run(inputs, trace=False)
    return out


# revision 16
# speedup vs baseline: 1.0195x; 1.0195x over previous
"""Trainium2 Bass kernel for nn_DGAD_net (vq_codebook).

Strategy
--------
The reference's dominant cost is ``einsum('bchw,oc->bohw', inter, w).mean((2,3))``
followed by tiny MLPs.  The einsum and the spatial mean commute (both linear),
so on device we only *sum-pool* the feature maps and fold ``w / (H*W)`` into the
first matmul — the problem becomes DMA/reduction bound instead of 100+ GFLOP of
conv.

Data parallel over 8 NeuronCores: batch 512 -> 64 per core.  Weights are
host-preprocessed (transposed / permuted / scaled / concat-split) and
replicated.  Each core returns a [1, 264] row of per-sample partials:
  [osvdd_main(64) | csvdd_main(64) | osvdd_aug(64) | csvdd_aug(64)
   | sum(q0*ls1)(4) | sum(q1*ls0)(4)]
and the host reduces those to the [4, 1] output.

Trace-driven tuning (v3, 357us vs 412us baseline):
 * Big tiles — 4-sample inter (3.2 MB) / 16-sample origin (1.6 MB) —
   alternating across the SP+ACT HWDGE rings: sustains ~418 GB/s
   through the whole stream (the old 400 KB origin DMAs were
   issue-bound at ~50% bandwidth).
 * Load order xi -> ai -> xo -> ao with phase-interleaved emission:
   branch-0's sfc chain is emitted mid-ai so its DVE ops never block
   the streaming pool reduces; the kernel tail is branch-1's MLP chains
   (PE-bound, ~60us of LDWEIGHTS+MATMUL pairs).
 * DVE reduce is PERF_ONE (~1.04 cy/elem regardless of dtype); a
   TensorTensor pre-fold halves the reduce itself but TT is also
   1 elem/lane/cycle here, so DVE stays ~118 Ge/s ≈ 262us total —
   just under the 279us DMA floor.
v6 (358us under heavy DVFS throttle, where v3 re-measured 426us): all
origin DMAs on the SP ring (no compute there, so issues gate only on
buffer sems) and svdd's (x-c)^2 as a single ACT Square-with-bias op —
together they stop late origin tiles/reduces from queueing behind the
branch-1 compute chains; the kernel now ends ~2us after the last DMA
byte.  Re-splitting origins across SP+ACT with DMA-first emission
regressed (v7, 392us): origin buffer sems still resolve behind the ai
reduces on saturated DVE, so ACT-ring issues straggle regardless.
Dead ends (measured): SWDGE f32->bf16 cast-DMA runs at only ~150-200
GB/s and bf16 does NOT speed up the PERF_ONE reduce (v2, 404us);
weights or late origin tiles on the gpsimd ring fire at t~0 (its engine
queue is empty, so issue order decouples from emission order) and
disrupt the stream (v4 410us / v5 420us).

The distill term only needs softmax/log_softmax of ``sim`` which are invariant
to per-row shifts, so the device computes ``score[b,k] = 2 t.p_k - ||p_k||^2``
(skipping ``-||t||^2``) — same softmax, same argmax.

This walrus build only encodes ONE sync wait per instruction; ``_split_waits``
rewrites the traced BIR, moving excess waits onto preceding same-engine NOPs.
"""

import sys

for _p in ("/opt/trn_rl_repo", "/root/.axon_site/_ro/trn_rl_repo"):
    if _p not in sys.path:
        sys.path.append(_p)

import numpy as np

B, CI, HW2, CO, HO2, D, DOM = 512, 256, 784, 512, 49, 64, 4
NCORE = 8
BC = B // NCORE  # 64 samples per core
EPOCHS = 30
W_TEMP, T_TEMP = 0.7, 0.4
_SCHED = np.concatenate(
    [np.linspace(W_TEMP, T_TEMP, int(EPOCHS * 0.25)),
     np.ones(EPOCHS - int(EPOCHS * 0.25)) * T_TEMP]
)

_NC = None  # built once per process
_ctr = [0]


def _split_waits(nc, mybir, cap=1):
    """Move excess sync waits onto preceding same-engine NOPs (this
    walrus encodes at most `cap` waits per instruction).  Same-engine
    program order preserves semantics exactly."""
    for f in nc.m.functions:
        for bb in f.blocks:
            new = []
            for inst in bb.instructions:
                si = inst.sync_info
                if si is not None and si.on_wait and len(si.on_wait) > cap:
                    waits = list(si.on_wait)
                    excess, keep = waits[:-cap], waits[-cap:]
                    while excess:
                        chunk, excess = excess[:cap], excess[cap:]
                        nop = mybir.InstNoOp(
                            name=f"I-wsplit-{_ctr[0]}", ins=[], outs=[]
                        )
                        _ctr[0] += 1
                        nop.engine = inst.engine
                        nop.sync_info = mybir.SyncInfo(on_wait=chunk, on_update=[])
                        new.append(nop)
                    inst.sync_info = mybir.SyncInfo(
                        on_wait=keep, on_update=list(si.on_update)
                    )
                new.append(inst)
            bb.instructions = new


def _build_nc():
    import concourse.bass as bass
    import concourse.tile as tile
    from concourse import mybir
    from contextlib import ExitStack

    AF = mybir.ActivationFunctionType
    AL = mybir.AluOpType
    AX = mybir.AxisListType
    f32 = mybir.dt.float32
    bf16 = mybir.dt.bfloat16

    nc = bass.Bass(trn_type="TRN2")

    # ---- DRAM I/O ----
    xi = nc.dram_tensor("xi", [BC, CI * HW2], f32, kind="ExternalInput")
    xo = nc.dram_tensor("xo", [BC, CO * HO2], f32, kind="ExternalInput")
    ai = nc.dram_tensor("ai", [BC, CI * HW2], f32, kind="ExternalInput")
    ao = nc.dram_tensor("ao", [BC, CO * HO2], f32, kind="ExternalInput")
    w_sw = nc.dram_tensor("w_sw", [2, 128, 512], bf16, kind="ExternalInput")
    w_s1 = nc.dram_tensor("w_s1", [4, 128, 1024], bf16, kind="ExternalInput")
    w_s2 = nc.dram_tensor("w_s2", [8, 128, 512], bf16, kind="ExternalInput")
    w_s3 = nc.dram_tensor("w_s3", [4, 128, 64], bf16, kind="ExternalInput")
    w_o1 = nc.dram_tensor("w_o1", [4, 128, 1024], bf16, kind="ExternalInput")
    w_o2 = nc.dram_tensor("w_o2", [8, 128, 512], bf16, kind="ExternalInput")
    w_o3 = nc.dram_tensor("w_o3", [4, 128, 64], bf16, kind="ExternalInput")
    w_t1 = nc.dram_tensor("w_t1", [64, 64], bf16, kind="ExternalInput")
    w_t2 = nc.dram_tensor("w_t2", [64, 64], bf16, kind="ExternalInput")
    w_c1 = nc.dram_tensor("w_c1", [64, 64], bf16, kind="ExternalInput")
    w_c2 = nc.dram_tensor("w_c2", [64, 64], bf16, kind="ExternalInput")
    w_q1 = nc.dram_tensor("w_q1", [64, 64], bf16, kind="ExternalInput")
    w_q2 = nc.dram_tensor("w_q2", [64, 64], bf16, kind="ExternalInput")
    w_pc = nc.dram_tensor("w_pc", [4, 64], bf16, kind="ExternalInput")
    w_p2 = nc.dram_tensor("w_p2", [65, 4], f32, kind="ExternalInput")
    b_t1 = nc.dram_tensor("b_t1", [64, 1], f32, kind="ExternalInput")
    b_cn = nc.dram_tensor("b_cn", [64, 1], f32, kind="ExternalInput")
    invt = nc.dram_tensor("invt", [64, 1], f32, kind="ExternalInput")
    idm = nc.dram_tensor("idm", [64, 64], f32, kind="ExternalInput")
    out = nc.dram_tensor("out", [1, 264], f32, kind="ExternalOutput")

    with tile.TileContext(nc) as tc:
        with ExitStack() as ctx:
            wp = ctx.enter_context(tc.tile_pool(name="wp", bufs=1))
            ip = ctx.enter_context(tc.tile_pool(name="ip", bufs=4))
            op = ctx.enter_context(tc.tile_pool(name="op", bufs=4))
            plp = ctx.enter_context(tc.tile_pool(name="plp", bufs=1))
            ap = ctx.enter_context(tc.tile_pool(name="ap", bufs=1))
            sp = ctx.enter_context(tc.tile_pool(name="sp", bufs=1))
            psA = ctx.enter_context(tc.tile_pool(name="psA", bufs=4, space="PSUM"))
            psB = ctx.enter_context(tc.tile_pool(name="psB", bufs=1, space="PSUM"))

            def wtile(h, shape, tag, rearr=None, dt=f32):
                t = wp.tile(shape, dt, tag=tag)
                src = h[:]
                if rearr:
                    src = src.rearrange(rearr)
                nc.scalar.dma_start(t[:], src)
                return t

            sw_sb = wtile(w_sw, [128, 2, 512], "w_sw", "m p j -> p m j", bf16)
            s1_sb = wtile(w_s1, [128, 4, 1024], "w_s1", "m p j -> p m j", bf16)
            s2_sb = wtile(w_s2, [128, 8, 512], "w_s2", "m p j -> p m j", bf16)
            s3_sb = wtile(w_s3, [128, 4, 64], "w_s3", "m p j -> p m j", bf16)
            o1_sb = wtile(w_o1, [128, 4, 1024], "w_o1", "m p j -> p m j", bf16)
            o2_sb = wtile(w_o2, [128, 8, 512], "w_o2", "m p j -> p m j", bf16)
            o3_sb = wtile(w_o3, [128, 4, 64], "w_o3", "m p j -> p m j", bf16)
            t1_sb = wtile(w_t1, [64, 64], "w_t1", dt=bf16)
            t2_sb = wtile(w_t2, [64, 64], "w_t2", dt=bf16)
            c1_sb = wtile(w_c1, [64, 64], "w_c1", dt=bf16)
            c2_sb = wtile(w_c2, [64, 64], "w_c2", dt=bf16)
            q1_sb = wtile(w_q1, [64, 64], "w_q1", dt=bf16)
            q2_sb = wtile(w_q2, [64, 64], "w_q2", dt=bf16)
            pc_sb = wtile(w_pc, [4, 64], "w_pc", dt=bf16)
            p2_sb = wtile(w_p2, [65, 4], "w_p2")
            bt1_sb = wtile(b_t1, [64, 1], "b_t1")
            bcn_sb = wtile(b_cn, [64, 1], "b_cn")
            it_sb = wtile(invt, [64, 1], "invt")
            id_sb = wtile(idm, [64, 64], "idm")

            ones_sb = wp.tile([64, 1], f32, tag="ones")
            nc.vector.memset(ones_sb[:], 1.0)
            out_sb = wp.tile([1, 264], f32, tag="out_sb")

            # ---- pooling ----
            # 4-sample inter tiles (3.2 MB) / 16-sample origin tiles
            # (1.6 MB), alternating SP / ACT HWDGE rings (two rings
            # together sustain ~415 GB/s; small tiles were issue-bound).
            # A TensorTensor pre-fold (PERF_TWO datapath, 2 out/lane/cy)
            # halves the element count before the PERF_ONE reduce.
            def pool_inter_phase(xh, dst, t0, t1):
                """tiles t0..t1-1 of 4 samples each.  dst: [128,2,BC] f32."""
                for ti in range(t0, t1):
                    b0 = ti * 4
                    t = ip.tile([128, 4, 2 * HW2], f32, tag="it")
                    eng = nc.sync if ti % 2 == 0 else nc.scalar
                    eng.dma_start(
                        t[:], xh[b0:b0 + 4].rearrange("b (p f) -> p b f", p=128)
                    )
                    a = t[:].rearrange("p b (g h) -> p b g h", g=2)
                    nc.vector.tensor_add(
                        a[:, :, :, 0:392], a[:, :, :, 0:392], a[:, :, :, 392:784]
                    )
                    nc.vector.reduce_sum(
                        dst[:, :, b0:b0 + 4].rearrange("p g b -> p b g"),
                        a[:, :, :, 0:392],
                        axis=AX.X,
                    )

            def pool_origin_phase(xh, dst):
                """dst: [128, 4, BC] f32.  4 tiles of 16 samples."""
                for ti in range(4):
                    b0 = ti * 16
                    t = op.tile([128, 16, 4 * HO2], f32, tag="ot")
                    # all origin tiles on the SP ring: its queue carries no
                    # compute, so these issues are gated only by buffer
                    # sems — on the ACT ring they queued behind sfc1/ofc0
                    # activation chains and landed ~40us late
                    eng = nc.sync
                    eng.dma_start(
                        t[:], xh[b0:b0 + 16].rearrange("b (p f) -> p b f", p=128)
                    )
                    nc.vector.reduce_sum(
                        dst[:, :, b0:b0 + 16].rearrange("p g b -> p b g"),
                        t[:].rearrange("p b (g h) -> p b g h", g=4),
                        axis=AX.X,
                    )

            def cast_pool(dst, tag, g):
                db = plp.tile([128, g, BC], bf16, tag=tag)
                nc.vector.tensor_copy(db[:], dst[:])
                return db

            # ---- MLP helpers ----
            def chunk_layer(w_sb, ins, nout_chunks, outw, r, nm, act=True):
                """outT chunks [outw, BC] = Lrelu( sum_m w_sb[:, m, chunk] @ ins[m] )."""
                outs = []
                nin = len(ins)
                for m2 in range(nout_chunks):
                    ps = psA.tile([128, BC], f32, tag="mm")
                    for m in range(nin):
                        nc.tensor.matmul(
                            ps[:outw, :],
                            w_sb[:, m, m2 * outw:(m2 + 1) * outw],
                            ins[m][:],
                            start=(m == 0),
                            stop=(m == nin - 1),
                        )
                    tl = ap.tile([outw, BC], bf16, tag=f"r{r}{nm}{m2}")
                    if act:
                        nc.scalar.activation(tl[:], ps[:outw, :], AF.Lrelu,
                                             alpha=0.01)
                    else:
                        nc.scalar.copy(tl[:], ps[:outw, :])
                    outs.append(tl)
                return outs

            def small_mm(lhsT, rhs, r, nm, act=True, bias=None, extra=None):
                """[64, BC] = act(lhsT.T @ rhs [+ extra matmul] + bias)."""
                ps = psA.tile([128, BC], f32, tag="mm")
                nc.tensor.matmul(
                    ps[:64, :], lhsT[:], rhs[:],
                    start=True, stop=(extra is None),
                )
                if extra is not None:
                    nc.tensor.matmul(
                        ps[:64, :], extra[0][:], extra[1][:], start=False, stop=True
                    )
                tl = ap.tile([64, BC], bf16, tag=f"r{r}{nm}")
                if bias is not None:
                    nc.scalar.activation(
                        tl[:], ps[:64, :], AF.Lrelu, bias=bias[:], alpha=0.01
                    )
                else:
                    nc.scalar.activation(tl[:], ps[:64, :], AF.Lrelu, alpha=0.01)
                return tl

            def svdd_row(featT, r, nm, off):
                """out_sb[0, off:off+64] = || featT[:, b] - center ||^2 per b.
                Square(x + (-c)) in ONE ACT op: keeps the ofc chains
                entirely off DVE so late origin-pool reduces are never
                queued behind them."""
                sq = sp.tile([64, BC], f32, tag=f"r{r}{nm}sq")
                nc.scalar.activation(sq[:], featT[:], AF.Square, bias=bcn_sb[:])
                pr = psB.tile([1, BC], f32, tag="row")
                nc.tensor.matmul(pr[:], ones_sb[:], sq[:])
                nc.scalar.copy(out_sb[:, off:off + BC], pr[:])

            # distill helpers
            def teacher_q(score, mx, r):
                nb = sp.tile([64, 1], f32, tag=f"nb{r}")
                nc.vector.tensor_scalar(nb[:], mx[:], it_sb[:], -1.0,
                                        op0=AL.mult, op1=AL.mult)
                e = sp.tile([64, 4], f32, tag=f"te{r}")
                es = sp.tile([64, 1], f32, tag=f"tes{r}")
                nc.scalar.activation(e[:], score[:], AF.Exp, bias=nb[:],
                                     scale=it_sb[:], accum_out=es[:])
                rc = sp.tile([64, 1], f32, tag=f"trc{r}")
                nc.vector.reciprocal(rc[:], es[:])
                q = sp.tile([64, 4], f32, tag=f"tq{r}")
                nc.vector.tensor_scalar(q[:], e[:], rc[:], None, op0=AL.mult)
                return q

            def student_ls(score, mx, r):
                nb = sp.tile([64, 1], f32, tag=f"snb{r}")
                nc.vector.tensor_scalar(nb[:], mx[:], -1.0, None, op0=AL.mult)
                e = sp.tile([64, 4], f32, tag=f"se{r}")
                es = sp.tile([64, 1], f32, tag=f"ses{r}")
                nc.scalar.activation(e[:], score[:], AF.Exp, bias=nb[:],
                                     accum_out=es[:])
                ln = sp.tile([64, 1], f32, tag=f"sln{r}")
                nc.scalar.activation(ln[:], es[:], AF.Ln)
                lse = sp.tile([64, 1], f32, tag=f"slse{r}")
                nc.vector.tensor_add(lse[:], ln[:], mx[:])
                ls = sp.tile([64, 4], f32, tag=f"sls{r}")
                nc.vector.tensor_scalar(ls[:], score[:], lse[:], None,
                                        op0=AL.subtract)
                return ls

            scores, qs, lss, ohTs = {}, {}, {}, {}

            def sfc_part(r, pib):
                """inter-dependent chain: shallow conv + sfc + texture +
                score + distill stats + onehot prototype pick."""
                sh = chunk_layer(sw_sb, [pib[:, g, :] for g in range(2)], 4,
                                 128, r, "sh", act=False)
                a1 = chunk_layer(s1_sb, sh, 8, 128, r, "a1")
                a2 = chunk_layer(s2_sb, a1, 4, 128, r, "a2")
                ps = psA.tile([128, BC], f32, tag="mm")
                for m in range(4):
                    nc.tensor.matmul(ps[:64, :], s3_sb[:, m, :], a2[m][:],
                                     start=(m == 0), stop=(m == 3))
                sT = ap.tile([64, BC], bf16, tag=f"r{r}sT")
                nc.scalar.activation(sT[:], ps[:64, :], AF.Lrelu, alpha=0.01)

                t1 = small_mm(t1_sb, sT, r, "t1", bias=bt1_sb)
                ps = psA.tile([128, BC], f32, tag="mm")
                nc.tensor.matmul(ps[:64, :], t2_sb[:], t1[:])
                tx = ap.tile([65, BC], f32, tag=f"r{r}tx")
                nc.scalar.activation(tx[0:64, :], ps[:64, :], AF.Lrelu, alpha=0.01)
                nc.vector.memset(tx[64:65, :], 1.0)

                pss = psB.tile([64, 4], f32, tag="sc")
                nc.tensor.matmul(pss[:], tx[:], p2_sb[:])
                score = sp.tile([64, 4], f32, tag=f"score{r}")
                nc.scalar.copy(score[:], pss[:])
                mx = sp.tile([64, 1], f32, tag=f"mx{r}")
                nc.vector.reduce_max(mx[:], score[:], axis=AX.X)
                scores[r] = (score, mx)
                qs[r] = teacher_q(score, mx, r)
                lss[r] = student_ls(score, mx, r)

                oh1 = sp.tile([64, 4], f32, tag=f"oh{r}")
                nc.vector.tensor_scalar(oh1[:], score[:], mx[:], None,
                                        op0=AL.is_ge)
                psT = psB.tile([4, 64], f32, tag="ohT")
                nc.tensor.transpose(psT[:], oh1[:], id_sb[:])
                ohT = sp.tile([4, 64], bf16, tag=f"ohT{r}")
                nc.scalar.copy(ohT[:], psT[:])
                ohTs[r] = ohT

            def ofc_part(r, pob):
                """origin-dependent chain: ofc + cfc + oc head + svdd rows."""
                b1 = chunk_layer(o1_sb, [pob[:, g, :] for g in range(4)], 8,
                                 128, r, "b1")
                b2 = chunk_layer(o2_sb, b1, 4, 128, r, "b2")
                ps = psA.tile([128, BC], f32, tag="mm")
                for m in range(4):
                    nc.tensor.matmul(ps[:64, :], o3_sb[:, m, :], b2[m][:],
                                     start=(m == 0), stop=(m == 3))
                orT = ap.tile([64, BC], bf16, tag=f"r{r}orT")
                nc.scalar.activation(orT[:], ps[:64, :], AF.Lrelu, alpha=0.01)

                cf1 = small_mm(c1_sb, orT, r, "cf1", extra=(pc_sb, ohTs[r]))
                clsT = small_mm(c2_sb, cf1, r, "cls")
                svdd_row(clsT, r, "c", off=(64 if r == 0 else 192))

                g1 = small_mm(q1_sb, orT, r, "g1")
                g2 = small_mm(q2_sb, g1, r, "g2")
                svdd_row(g2, r, "o", off=(0 if r == 0 else 128))

            # ================= emission schedule =================
            pi0 = plp.tile([128, 2, BC], f32, tag="pi0")
            pi1 = plp.tile([128, 2, BC], f32, tag="pi1")
            po0 = plp.tile([128, 4, BC], f32, tag="po0")
            po1 = plp.tile([128, 4, BC], f32, tag="po1")

            # xi
            pool_inter_phase(xi, pi0, 0, 16)
            pi0b = cast_pool(pi0, "pi0b", 2)
            # ai first half
            pool_inter_phase(ai, pi1, 0, 8)
            # branch-0 inter chain (its DVE ops land behind ai[0:8] reduces)
            sfc_part(0, pi0b)
            # ai second half
            pool_inter_phase(ai, pi1, 8, 16)
            pi1b = cast_pool(pi1, "pi1b", 2)
            # branch-1 inter chain (PE/ACT start as soon as pi1b ready)
            sfc_part(1, pi1b)
            # xo
            pool_origin_phase(xo, po0)
            po0b = cast_pool(po0, "po0b", 4)
            ofc_part(0, po0b)
            # ao
            pool_origin_phase(ao, po1)
            po1b = cast_pool(po1, "po1b", 4)
            ofc_part(1, po1b)

            # distill cross terms
            pr01 = sp.tile([64, 4], f32, tag="pr01")
            nc.vector.tensor_mul(pr01[:], qs[0][:], lss[1][:])
            pc01 = psB.tile([1, 4], f32, tag="pc")
            nc.tensor.matmul(pc01[:], ones_sb[:], pr01[:])
            nc.scalar.copy(out_sb[:, 256:260], pc01[:])

            pr10 = sp.tile([64, 4], f32, tag="pr10")
            nc.vector.tensor_mul(pr10[:], qs[1][:], lss[0][:])
            pc10 = psB.tile([1, 4], f32, tag="pc")
            nc.tensor.matmul(pc10[:], ones_sb[:], pr10[:])
            nc.scalar.copy(out_sb[:, 260:264], pc10[:])

            nc.sync.dma_start(out[:], out_sb[:])

    _split_waits(nc, mybir)
    return nc


def _get_nc():
    global _NC
    if _NC is None:
        _NC = _build_nc()
    return _NC


def _prep_weights(shallow_conv_w, ofc_w1, ofc_w2, ofc_w3, sfc_w1, sfc_w2, sfc_w3,
                  tfc_w1, tfc_w2, cfc_w1, cfc_w2, oc_w1, oc_w2, center, protos,
                  epoch):
    f = np.float32
    sw = np.asarray(shallow_conv_w, f)
    o1, o2, o3 = (np.asarray(a, f) for a in (ofc_w1, ofc_w2, ofc_w3))
    s1, s2, s3 = (np.asarray(a, f) for a in (sfc_w1, sfc_w2, sfc_w3))
    t1, t2 = np.asarray(tfc_w1, f), np.asarray(tfc_w2, f)
    c1, c2 = np.asarray(cfc_w1, f), np.asarray(cfc_w2, f)
    q1, q2 = np.asarray(oc_w1, f), np.asarray(oc_w2, f)
    ctr = np.asarray(center, f)
    pr = np.asarray(protos, f)

    import ml_dtypes
    bf = ml_dtypes.bfloat16

    w = {}
    # channel c = 2p+g for inter (1568 floats per partition), 4p+g for origin
    w["w_sw"] = np.ascontiguousarray(
        (sw.T / HW2).astype(f).reshape(128, 2, 512).transpose(1, 0, 2)).astype(bf)
    w["w_s1"] = np.ascontiguousarray(s1.T.reshape(4, 128, 1024)).astype(bf)
    w["w_s2"] = np.ascontiguousarray(s2.T.reshape(8, 128, 512)).astype(bf)
    w["w_s3"] = np.ascontiguousarray(s3.T.reshape(4, 128, 64)).astype(bf)
    w["w_o1"] = np.ascontiguousarray(
        (o1.T / HO2).astype(f).reshape(128, 4, 1024).transpose(1, 0, 2)).astype(bf)
    w["w_o2"] = np.ascontiguousarray(o2.T.reshape(8, 128, 512)).astype(bf)
    w["w_o3"] = np.ascontiguousarray(o3.T.reshape(4, 128, 64)).astype(bf)
    ta, tb = t1[:, :64], t1[:, 64:]
    w["w_t1"] = np.ascontiguousarray((ta + tb).T).astype(bf)
    w["b_t1"] = np.ascontiguousarray(-(tb @ ctr))[:, None]
    w["w_t2"] = np.ascontiguousarray(t2.T).astype(bf)
    ca, cb = c1[:, :64], c1[:, 64:]
    w["w_c1"] = np.ascontiguousarray((ca + cb).T).astype(bf)
    w["w_pc"] = np.ascontiguousarray(-(pr @ cb.T)).astype(bf)
    w["w_c2"] = np.ascontiguousarray(c2.T).astype(bf)
    w["w_q1"] = np.ascontiguousarray(q1.T).astype(bf)
    w["w_q2"] = np.ascontiguousarray(q2.T).astype(bf)
    p2 = np.concatenate([2.0 * pr.T, -(pr ** 2).sum(1)[None, :]], 0).astype(f)
    w["w_p2"] = np.ascontiguousarray(p2)
    w["b_cn"] = np.ascontiguousarray(-ctr)[:, None]
    temp = f(_SCHED[int(np.asarray(epoch))])
    w["invt"] = np.full((64, 1), 1.0 / temp, f)
    w["idm"] = np.eye(64, dtype=f)
    return w


def _run(inputs, trace=False):
    from concourse.bass_utils import run_bass_kernel_spmd

    nc = _get_nc()
    f = np.float32
    inter = np.asarray(inputs["inter_feat"], f).reshape(B, CI * HW2)
    orig = np.asarray(inputs["origin_feat"], f).reshape(B, CO * HO2)
    ainter = np.asarray(inputs["aug_inter_feat"], f).reshape(B, CI * HW2)
    aorig = np.asarray(inputs["aug_origin_feat"], f).reshape(B, CO * HO2)
    w = _prep_weights(
        inputs["shallow_conv_w"], inputs["ofc_w1"], inputs["ofc_w2"],
        inputs["ofc_w3"], inputs["sfc_w1"], inputs["sfc_w2"], inputs["sfc_w3"],
        inputs["tfc_w1"], inputs["tfc_w2"], inputs["cfc_w1"], inputs["cfc_w2"],
        inputs["oc_w1"], inputs["oc_w2"], inputs["center"], inputs["protos"],
        inputs["epoch"],
    )
    in_maps = []
    for c in range(NCORE):
        sl = slice(c * BC, (c + 1) * BC)
        m = dict(w)
        m["xi"] = np.ascontiguousarray(inter[sl])
        m["xo"] = np.ascontiguousarray(orig[sl])
        m["ai"] = np.ascontiguousarray(ainter[sl])
        m["ao"] = np.ascontiguousarray(aorig[sl])
        in_maps.append(m)

    res = run_bass_kernel_spmd(nc, in_maps, core_ids=list(range(NCORE)),
                               trace=trace)
    rows = np.stack([res.results[c]["out"][0] for c in range(NCORE)])  # [8, 264]
    osv0 = rows[:, 0:64].astype(f)
    csv0 = rows[:, 64:128].astype(f)
    osv1 = rows[:, 128:192].astype(f)
    csv1 = rows[:, 192:256].astype(f)
    s01 = rows[:, 256:260].astype(f)
    s10 = rows[:, 260:264].astype(f)

    l01 = f(-(s01.sum(dtype=f)) / B)
    l10 = f(-(s10.sum(dtype=f)) / B)
    distill = f((l01 + l10) / 2.0)
    row_o = f(osv0.sum(dtype=f) / B + osv1.sum(dtype=f) / B)
    row_c = f(csv0.sum(dtype=f) / B + csv1.sum(dtype=f) / B)
    row_a = f(np.abs(osv0 - csv0).sum(dtype=f) / B
              + np.abs(osv1 - csv1).sum(dtype=f) / B)
    out = np.array([[distill], [row_o], [row_c], [row_a]], dtype=f)
    return out, res


def kernel(**inputs):
    out, _ = _run(inputs, trace=False)
    return out


# revision 17
# speedup vs baseline: 1.0932x; 1.0723x over previous
"""Trainium2 Bass kernel for nn_DGAD_net (vq_codebook).

Strategy
--------
The reference's dominant cost is ``einsum('bchw,oc->bohw', inter, w).mean((2,3))``
followed by tiny MLPs.  The einsum and the spatial mean commute (both linear),
so on device we only *sum-pool* the feature maps and fold ``w / (H*W)`` into the
first matmul — the problem becomes DMA/reduction bound instead of 100+ GFLOP of
conv.

Data parallel over 8 NeuronCores: batch 512 -> 64 per core.  Weights are
host-preprocessed (transposed / permuted / scaled / concat-split) and
replicated.  Each core returns a [1, 264] row of per-sample partials:
  [osvdd_main(64) | csvdd_main(64) | osvdd_aug(64) | csvdd_aug(64)
   | sum(q0*ls1)(4) | sum(q1*ls0)(4)]
and the host reduces those to the [4, 1] output.

Trace-driven tuning (v3, 357us vs 412us baseline):
 * Big tiles — 4-sample inter (3.2 MB) / 16-sample origin (1.6 MB) —
   alternating across the SP+ACT HWDGE rings: sustains ~418 GB/s
   through the whole stream (the old 400 KB origin DMAs were
   issue-bound at ~50% bandwidth).
 * Load order xi -> ai -> xo -> ao with phase-interleaved emission:
   branch-0's sfc chain is emitted mid-ai so its DVE ops never block
   the streaming pool reduces; the kernel tail is branch-1's MLP chains
   (PE-bound, ~60us of LDWEIGHTS+MATMUL pairs).
 * DVE reduce is PERF_ONE (~1.04 cy/elem regardless of dtype); a
   TensorTensor pre-fold halves the reduce itself but TT is also
   1 elem/lane/cycle here, so DVE stays ~118 Ge/s ≈ 262us total —
   just under the 279us DMA floor.
Dead ends (measured): SWDGE f32->bf16 cast-DMA runs at only ~150-200
GB/s and bf16 does NOT speed up the PERF_ONE reduce (v2, 404us);
weights or late origin tiles on the gpsimd ring fire at t~0 (its engine
queue is empty, so issue order decouples from emission order) and
disrupt the stream (v4 410us / v5 420us).

The distill term only needs softmax/log_softmax of ``sim`` which are invariant
to per-row shifts, so the device computes ``score[b,k] = 2 t.p_k - ||p_k||^2``
(skipping ``-||t||^2``) — same softmax, same argmax.

This walrus build only encodes ONE sync wait per instruction; ``_split_waits``
rewrites the traced BIR, moving excess waits onto preceding same-engine NOPs.
"""

import sys

for _p in ("/opt/trn_rl_repo", "/root/.axon_site/_ro/trn_rl_repo"):
    if _p not in sys.path:
        sys.path.append(_p)

import numpy as np

B, CI, HW2, CO, HO2, D, DOM = 512, 256, 784, 512, 49, 64, 4
NCORE = 8
BC = B // NCORE  # 64 samples per core
EPOCHS = 30
W_TEMP, T_TEMP = 0.7, 0.4
_SCHED = np.concatenate(
    [np.linspace(W_TEMP, T_TEMP, int(EPOCHS * 0.25)),
     np.ones(EPOCHS - int(EPOCHS * 0.25)) * T_TEMP]
)

_NC = None  # built once per process
_ctr = [0]


def _split_waits(nc, mybir, cap=1):
    """Move excess sync waits onto preceding same-engine NOPs (this
    walrus encodes at most `cap` waits per instruction).  Same-engine
    program order preserves semantics exactly."""
    for f in nc.m.functions:
        for bb in f.blocks:
            new = []
            for inst in bb.instructions:
                si = inst.sync_info
                if si is not None and si.on_wait and len(si.on_wait) > cap:
                    waits = list(si.on_wait)
                    excess, keep = waits[:-cap], waits[-cap:]
                    while excess:
                        chunk, excess = excess[:cap], excess[cap:]
                        nop = mybir.InstNoOp(
                            name=f"I-wsplit-{_ctr[0]}", ins=[], outs=[]
                        )
                        _ctr[0] += 1
                        nop.engine = inst.engine
                        nop.sync_info = mybir.SyncInfo(on_wait=chunk, on_update=[])
                        new.append(nop)
                    inst.sync_info = mybir.SyncInfo(
                        on_wait=keep, on_update=list(si.on_update)
                    )
                new.append(inst)
            bb.instructions = new


def _build_nc():
    import concourse.bass as bass
    import concourse.tile as tile
    from concourse import mybir
    from contextlib import ExitStack

    AF = mybir.ActivationFunctionType
    AL = mybir.AluOpType
    AX = mybir.AxisListType
    f32 = mybir.dt.float32
    bf16 = mybir.dt.bfloat16

    nc = bass.Bass(trn_type="TRN2")

    # ---- DRAM I/O ----
    xi = nc.dram_tensor("xi", [BC, CI * HW2], f32, kind="ExternalInput")
    xo = nc.dram_tensor("xo", [BC, CO * HO2], f32, kind="ExternalInput")
    ai = nc.dram_tensor("ai", [BC, CI * HW2], f32, kind="ExternalInput")
    ao = nc.dram_tensor("ao", [BC, CO * HO2], f32, kind="ExternalInput")
    w_sw = nc.dram_tensor("w_sw", [2, 128, 512], bf16, kind="ExternalInput")
    w_s1 = nc.dram_tensor("w_s1", [4, 128, 1024], bf16, kind="ExternalInput")
    w_s2 = nc.dram_tensor("w_s2", [8, 128, 512], bf16, kind="ExternalInput")
    w_s3 = nc.dram_tensor("w_s3", [4, 128, 64], bf16, kind="ExternalInput")
    w_o1 = nc.dram_tensor("w_o1", [4, 128, 1024], bf16, kind="ExternalInput")
    w_o2 = nc.dram_tensor("w_o2", [8, 128, 512], bf16, kind="ExternalInput")
    w_o3 = nc.dram_tensor("w_o3", [4, 128, 64], bf16, kind="ExternalInput")
    w_t1 = nc.dram_tensor("w_t1", [64, 64], bf16, kind="ExternalInput")
    w_t2 = nc.dram_tensor("w_t2", [64, 64], bf16, kind="ExternalInput")
    w_c1 = nc.dram_tensor("w_c1", [64, 64], bf16, kind="ExternalInput")
    w_c2 = nc.dram_tensor("w_c2", [64, 64], bf16, kind="ExternalInput")
    w_q1 = nc.dram_tensor("w_q1", [64, 64], bf16, kind="ExternalInput")
    w_q2 = nc.dram_tensor("w_q2", [64, 64], bf16, kind="ExternalInput")
    w_pc = nc.dram_tensor("w_pc", [4, 64], bf16, kind="ExternalInput")
    w_p2 = nc.dram_tensor("w_p2", [65, 4], f32, kind="ExternalInput")
    b_t1 = nc.dram_tensor("b_t1", [64, 1], f32, kind="ExternalInput")
    b_cn = nc.dram_tensor("b_cn", [64, 1], f32, kind="ExternalInput")
    invt = nc.dram_tensor("invt", [64, 1], f32, kind="ExternalInput")
    idm = nc.dram_tensor("idm", [64, 64], f32, kind="ExternalInput")
    out = nc.dram_tensor("out", [1, 264], f32, kind="ExternalOutput")

    with tile.TileContext(nc) as tc:
        with ExitStack() as ctx:
            wp = ctx.enter_context(tc.tile_pool(name="wp", bufs=1))
            ip = ctx.enter_context(tc.tile_pool(name="ip", bufs=4))
            op = ctx.enter_context(tc.tile_pool(name="op", bufs=3))
            plp = ctx.enter_context(tc.tile_pool(name="plp", bufs=1))
            ap = ctx.enter_context(tc.tile_pool(name="ap", bufs=1))
            sp = ctx.enter_context(tc.tile_pool(name="sp", bufs=1))
            psA = ctx.enter_context(tc.tile_pool(name="psA", bufs=4, space="PSUM"))
            psB = ctx.enter_context(tc.tile_pool(name="psB", bufs=1, space="PSUM"))

            def wtile(h, shape, tag, rearr=None, dt=f32):
                t = wp.tile(shape, dt, tag=tag)
                src = h[:]
                if rearr:
                    src = src.rearrange(rearr)
                nc.scalar.dma_start(t[:], src)
                return t

            sw_sb = wtile(w_sw, [128, 2, 512], "w_sw", "m p j -> p m j", bf16)
            s1_sb = wtile(w_s1, [128, 4, 1024], "w_s1", "m p j -> p m j", bf16)
            s2_sb = wtile(w_s2, [128, 8, 512], "w_s2", "m p j -> p m j", bf16)
            s3_sb = wtile(w_s3, [128, 4, 64], "w_s3", "m p j -> p m j", bf16)
            o1_sb = wtile(w_o1, [128, 4, 1024], "w_o1", "m p j -> p m j", bf16)
            o2_sb = wtile(w_o2, [128, 8, 512], "w_o2", "m p j -> p m j", bf16)
            o3_sb = wtile(w_o3, [128, 4, 64], "w_o3", "m p j -> p m j", bf16)
            t1_sb = wtile(w_t1, [64, 64], "w_t1", dt=bf16)
            t2_sb = wtile(w_t2, [64, 64], "w_t2", dt=bf16)
            c1_sb = wtile(w_c1, [64, 64], "w_c1", dt=bf16)
            c2_sb = wtile(w_c2, [64, 64], "w_c2", dt=bf16)
            q1_sb = wtile(w_q1, [64, 64], "w_q1", dt=bf16)
            q2_sb = wtile(w_q2, [64, 64], "w_q2", dt=bf16)
            pc_sb = wtile(w_pc, [4, 64], "w_pc", dt=bf16)
            p2_sb = wtile(w_p2, [65, 4], "w_p2")
            bt1_sb = wtile(b_t1, [64, 1], "b_t1")
            bcn_sb = wtile(b_cn, [64, 1], "b_cn")
            it_sb = wtile(invt, [64, 1], "invt")
            id_sb = wtile(idm, [64, 64], "idm")

            ones_sb = wp.tile([64, 1], f32, tag="ones")
            nc.vector.memset(ones_sb[:], 1.0)
            out_sb = wp.tile([1, 264], f32, tag="out_sb")

            # ---- pooling ----
            # 4-sample inter tiles (3.2 MB) / 16-sample origin tiles
            # (1.6 MB), alternating SP / ACT HWDGE rings (two rings
            # together sustain ~415 GB/s; small tiles were issue-bound).
            # A TensorTensor pre-fold (PERF_TWO datapath, 2 out/lane/cy)
            # halves the element count before the PERF_ONE reduce.
            def pool_inter_phase(xh, dst, t0, t1):
                """tiles t0..t1-1 of 4 samples each.  dst: [128,2,BC] f32."""
                for ti in range(t0, t1):
                    b0 = ti * 4
                    t = ip.tile([128, 4, 2 * HW2], f32, tag="it")
                    eng = nc.sync if ti % 2 == 0 else nc.scalar
                    eng.dma_start(
                        t[:], xh[b0:b0 + 4].rearrange("b (p f) -> p b f", p=128)
                    )
                    a = t[:].rearrange("p b (g h) -> p b g h", g=2)
                    nc.vector.tensor_add(
                        a[:, :, :, 0:392], a[:, :, :, 0:392], a[:, :, :, 392:784]
                    )
                    nc.vector.reduce_sum(
                        dst[:, :, b0:b0 + 4].rearrange("p g b -> p b g"),
                        a[:, :, :, 0:392],
                        axis=AX.X,
                    )

            def pool_origin_phase(xh, dst):
                """dst: [128, 4, BC] f32.  4 tiles of 16 samples."""
                for ti in range(4):
                    b0 = ti * 16
                    t = op.tile([128, 16, 4 * HO2], f32, tag="ot")
                    # all origin tiles on the SP ring: its queue carries no
                    # compute, so these issues are gated only by buffer
                    # sems — on the ACT ring they queued behind sfc1/ofc0
                    # activation chains and landed ~40us late
                    eng = nc.sync
                    eng.dma_start(
                        t[:], xh[b0:b0 + 16].rearrange("b (p f) -> p b f", p=128)
                    )
                    nc.vector.reduce_sum(
                        dst[:, :, b0:b0 + 16].rearrange("p g b -> p b g"),
                        t[:].rearrange("p b (g h) -> p b g h", g=4),
                        axis=AX.X,
                    )

            def cast_pool(dst, tag, g):
                db = plp.tile([128, g, BC], bf16, tag=tag)
                nc.vector.tensor_copy(db[:], dst[:])
                return db

            # ---- MLP helpers ----
            def chunk_layer(w_sb, ins, nout_chunks, outw, r, nm, act=True):
                """outT chunks [outw, BC] = Lrelu( sum_m w_sb[:, m, chunk] @ ins[m] )."""
                outs = []
                nin = len(ins)
                for m2 in range(nout_chunks):
                    ps = psA.tile([128, BC], f32, tag="mm")
                    for m in range(nin):
                        nc.tensor.matmul(
                            ps[:outw, :],
                            w_sb[:, m, m2 * outw:(m2 + 1) * outw],
                            ins[m][:],
                            start=(m == 0),
                            stop=(m == nin - 1),
                        )
                    tl = ap.tile([outw, BC], bf16, tag=f"r{r}{nm}{m2}")
                    if act:
                        nc.scalar.activation(tl[:], ps[:outw, :], AF.Lrelu,
                                             alpha=0.01)
                    else:
                        nc.scalar.copy(tl[:], ps[:outw, :])
                    outs.append(tl)
                return outs

            def small_mm(lhsT, rhs, r, nm, act=True, bias=None, extra=None):
                """[64, BC] = act(lhsT.T @ rhs [+ extra matmul] + bias)."""
                ps = psA.tile([128, BC], f32, tag="mm")
                nc.tensor.matmul(
                    ps[:64, :], lhsT[:], rhs[:],
                    start=True, stop=(extra is None),
                )
                if extra is not None:
                    nc.tensor.matmul(
                        ps[:64, :], extra[0][:], extra[1][:], start=False, stop=True
                    )
                tl = ap.tile([64, BC], bf16, tag=f"r{r}{nm}")
                if bias is not None:
                    nc.scalar.activation(
                        tl[:], ps[:64, :], AF.Lrelu, bias=bias[:], alpha=0.01
                    )
                else:
                    nc.scalar.activation(tl[:], ps[:64, :], AF.Lrelu, alpha=0.01)
                return tl

            def svdd_row(featT, r, nm, off):
                """out_sb[0, off:off+64] = || featT[:, b] - center ||^2 per b.
                Square(x + (-c)) in ONE ACT op: keeps the ofc chains
                entirely off DVE so late origin-pool reduces are never
                queued behind them."""
                sq = sp.tile([64, BC], f32, tag=f"r{r}{nm}sq")
                nc.scalar.activation(sq[:], featT[:], AF.Square, bias=bcn_sb[:])
                pr = psB.tile([1, BC], f32, tag="row")
                nc.tensor.matmul(pr[:], ones_sb[:], sq[:])
                nc.scalar.copy(out_sb[:, off:off + BC], pr[:])

            # distill helpers
            def teacher_q(score, mx, r):
                nb = sp.tile([64, 1], f32, tag=f"nb{r}")
                nc.vector.tensor_scalar(nb[:], mx[:], it_sb[:], -1.0,
                                        op0=AL.mult, op1=AL.mult)
                e = sp.tile([64, 4], f32, tag=f"te{r}")
                es = sp.tile([64, 1], f32, tag=f"tes{r}")
                nc.scalar.activation(e[:], score[:], AF.Exp, bias=nb[:],
                                     scale=it_sb[:], accum_out=es[:])
                rc = sp.tile([64, 1], f32, tag=f"trc{r}")
                nc.vector.reciprocal(rc[:], es[:])
                q = sp.tile([64, 4], f32, tag=f"tq{r}")
                nc.vector.tensor_scalar(q[:], e[:], rc[:], None, op0=AL.mult)
                return q

            def student_ls(score, mx, r):
                nb = sp.tile([64, 1], f32, tag=f"snb{r}")
                nc.vector.tensor_scalar(nb[:], mx[:], -1.0, None, op0=AL.mult)
                e = sp.tile([64, 4], f32, tag=f"se{r}")
                es = sp.tile([64, 1], f32, tag=f"ses{r}")
                nc.scalar.activation(e[:], score[:], AF.Exp, bias=nb[:],
                                     accum_out=es[:])
                ln = sp.tile([64, 1], f32, tag=f"sln{r}")
                nc.scalar.activation(ln[:], es[:], AF.Ln)
                lse = sp.tile([64, 1], f32, tag=f"slse{r}")
                nc.vector.tensor_add(lse[:], ln[:], mx[:])
                ls = sp.tile([64, 4], f32, tag=f"sls{r}")
                nc.vector.tensor_scalar(ls[:], score[:], lse[:], None,
                                        op0=AL.subtract)
                return ls

            scores, qs, lss, ohTs = {}, {}, {}, {}

            def sfc_part(r, pib):
                """inter-dependent chain: shallow conv + sfc + texture +
                score + distill stats + onehot prototype pick."""
                sh = chunk_layer(sw_sb, [pib[:, g, :] for g in range(2)], 4,
                                 128, r, "sh", act=False)
                a1 = chunk_layer(s1_sb, sh, 8, 128, r, "a1")
                a2 = chunk_layer(s2_sb, a1, 4, 128, r, "a2")
                ps = psA.tile([128, BC], f32, tag="mm")
                for m in range(4):
                    nc.tensor.matmul(ps[:64, :], s3_sb[:, m, :], a2[m][:],
                                     start=(m == 0), stop=(m == 3))
                sT = ap.tile([64, BC], bf16, tag=f"r{r}sT")
                nc.scalar.activation(sT[:], ps[:64, :], AF.Lrelu, alpha=0.01)

                t1 = small_mm(t1_sb, sT, r, "t1", bias=bt1_sb)
                ps = psA.tile([128, BC], f32, tag="mm")
                nc.tensor.matmul(ps[:64, :], t2_sb[:], t1[:])
                tx = ap.tile([65, BC], f32, tag=f"r{r}tx")
                nc.scalar.activation(tx[0:64, :], ps[:64, :], AF.Lrelu, alpha=0.01)
                nc.vector.memset(tx[64:65, :], 1.0)

                pss = psB.tile([64, 4], f32, tag="sc")
                nc.tensor.matmul(pss[:], tx[:], p2_sb[:])
                score = sp.tile([64, 4], f32, tag=f"score{r}")
                nc.scalar.copy(score[:], pss[:])
                mx = sp.tile([64, 1], f32, tag=f"mx{r}")
                nc.vector.reduce_max(mx[:], score[:], axis=AX.X)
                scores[r] = (score, mx)
                qs[r] = teacher_q(score, mx, r)
                lss[r] = student_ls(score, mx, r)

                oh1 = sp.tile([64, 4], f32, tag=f"oh{r}")
                nc.vector.tensor_scalar(oh1[:], score[:], mx[:], None,
                                        op0=AL.is_ge)
                psT = psB.tile([4, 64], f32, tag="ohT")
                nc.tensor.transpose(psT[:], oh1[:], id_sb[:])
                ohT = sp.tile([4, 64], bf16, tag=f"ohT{r}")
                nc.scalar.copy(ohT[:], psT[:])
                ohTs[r] = ohT

            def ofc_part(r, pob):
                """origin-dependent chain: ofc + cfc + oc head + svdd rows."""
                b1 = chunk_layer(o1_sb, [pob[:, g, :] for g in range(4)], 8,
                                 128, r, "b1")
                b2 = chunk_layer(o2_sb, b1, 4, 128, r, "b2")
                ps = psA.tile([128, BC], f32, tag="mm")
                for m in range(4):
                    nc.tensor.matmul(ps[:64, :], o3_sb[:, m, :], b2[m][:],
                                     start=(m == 0), stop=(m == 3))
                orT = ap.tile([64, BC], bf16, tag=f"r{r}orT")
                nc.scalar.activation(orT[:], ps[:64, :], AF.Lrelu, alpha=0.01)

                cf1 = small_mm(c1_sb, orT, r, "cf1", extra=(pc_sb, ohTs[r]))
                clsT = small_mm(c2_sb, cf1, r, "cls")
                svdd_row(clsT, r, "c", off=(64 if r == 0 else 192))

                g1 = small_mm(q1_sb, orT, r, "g1")
                g2 = small_mm(q2_sb, g1, r, "g2")
                svdd_row(g2, r, "o", off=(0 if r == 0 else 128))

            # ================= emission schedule =================
            pi0 = plp.tile([128, 2, BC], f32, tag="pi0")
            pi1 = plp.tile([128, 2, BC], f32, tag="pi1")
            po0 = plp.tile([128, 4, BC], f32, tag="po0")
            po1 = plp.tile([128, 4, BC], f32, tag="po1")

            # xi
            pool_inter_phase(xi, pi0, 0, 16)
            pi0b = cast_pool(pi0, "pi0b", 2)
            # ai first half
            pool_inter_phase(ai, pi1, 0, 8)
            # branch-0 inter chain (its DVE ops land behind ai[0:8] reduces)
            sfc_part(0, pi0b)
            # ai second half
            pool_inter_phase(ai, pi1, 8, 16)
            pi1b = cast_pool(pi1, "pi1b", 2)
            # branch-1 inter chain (PE/ACT start as soon as pi1b ready)
            sfc_part(1, pi1b)
            # xo
            pool_origin_phase(xo, po0)
            po0b = cast_pool(po0, "po0b", 4)
            ofc_part(0, po0b)
            # ao
            pool_origin_phase(ao, po1)
            po1b = cast_pool(po1, "po1b", 4)
            ofc_part(1, po1b)

            # distill cross terms
            pr01 = sp.tile([64, 4], f32, tag="pr01")
            nc.vector.tensor_mul(pr01[:], qs[0][:], lss[1][:])
            pc01 = psB.tile([1, 4], f32, tag="pc")
            nc.tensor.matmul(pc01[:], ones_sb[:], pr01[:])
            nc.scalar.copy(out_sb[:, 256:260], pc01[:])

            pr10 = sp.tile([64, 4], f32, tag="pr10")
            nc.vector.tensor_mul(pr10[:], qs[1][:], lss[0][:])
            pc10 = psB.tile([1, 4], f32, tag="pc")
            nc.tensor.matmul(pc10[:], ones_sb[:], pr10[:])
            nc.scalar.copy(out_sb[:, 260:264], pc10[:])

            nc.sync.dma_start(out[:], out_sb[:])

    _split_waits(nc, mybir)
    return nc


def _get_nc():
    global _NC
    if _NC is None:
        _NC = _build_nc()
    return _NC


def _prep_weights(shallow_conv_w, ofc_w1, ofc_w2, ofc_w3, sfc_w1, sfc_w2, sfc_w3,
                  tfc_w1, tfc_w2, cfc_w1, cfc_w2, oc_w1, oc_w2, center, protos,
                  epoch):
    f = np.float32
    sw = np.asarray(shallow_conv_w, f)
    o1, o2, o3 = (np.asarray(a, f) for a in (ofc_w1, ofc_w2, ofc_w3))
    s1, s2, s3 = (np.asarray(a, f) for a in (sfc_w1, sfc_w2, sfc_w3))
    t1, t2 = np.asarray(tfc_w1, f), np.asarray(tfc_w2, f)
    c1, c2 = np.asarray(cfc_w1, f), np.asarray(cfc_w2, f)
    q1, q2 = np.asarray(oc_w1, f), np.asarray(oc_w2, f)
    ctr = np.asarray(center, f)
    pr = np.asarray(protos, f)

    import ml_dtypes
    bf = ml_dtypes.bfloat16

    w = {}
    # channel c = 2p+g for inter (1568 floats per partition), 4p+g for origin
    w["w_sw"] = np.ascontiguousarray(
        (sw.T / HW2).astype(f).reshape(128, 2, 512).transpose(1, 0, 2)).astype(bf)
    w["w_s1"] = np.ascontiguousarray(s1.T.reshape(4, 128, 1024)).astype(bf)
    w["w_s2"] = np.ascontiguousarray(s2.T.reshape(8, 128, 512)).astype(bf)
    w["w_s3"] = np.ascontiguousarray(s3.T.reshape(4, 128, 64)).astype(bf)
    w["w_o1"] = np.ascontiguousarray(
        (o1.T / HO2).astype(f).reshape(128, 4, 1024).transpose(1, 0, 2)).astype(bf)
    w["w_o2"] = np.ascontiguousarray(o2.T.reshape(8, 128, 512)).astype(bf)
    w["w_o3"] = np.ascontiguousarray(o3.T.reshape(4, 128, 64)).astype(bf)
    ta, tb = t1[:, :64], t1[:, 64:]
    w["w_t1"] = np.ascontiguousarray((ta + tb).T).astype(bf)
    w["b_t1"] = np.ascontiguousarray(-(tb @ ctr))[:, None]
    w["w_t2"] = np.ascontiguousarray(t2.T).astype(bf)
    ca, cb = c1[:, :64], c1[:, 64:]
    w["w_c1"] = np.ascontiguousarray((ca + cb).T).astype(bf)
    w["w_pc"] = np.ascontiguousarray(-(pr @ cb.T)).astype(bf)
    w["w_c2"] = np.ascontiguousarray(c2.T).astype(bf)
    w["w_q1"] = np.ascontiguousarray(q1.T).astype(bf)
    w["w_q2"] = np.ascontiguousarray(q2.T).astype(bf)
    p2 = np.concatenate([2.0 * pr.T, -(pr ** 2).sum(1)[None, :]], 0).astype(f)
    w["w_p2"] = np.ascontiguousarray(p2)
    w["b_cn"] = np.ascontiguousarray(-ctr)[:, None]
    temp = f(_SCHED[int(np.asarray(epoch))])
    w["invt"] = np.full((64, 1), 1.0 / temp, f)
    w["idm"] = np.eye(64, dtype=f)
    return w


def _run(inputs, trace=False):
    from concourse.bass_utils import run_bass_kernel_spmd

    nc = _get_nc()
    f = np.float32
    inter = np.asarray(inputs["inter_feat"], f).reshape(B, CI * HW2)
    orig = np.asarray(inputs["origin_feat"], f).reshape(B, CO * HO2)
    ainter = np.asarray(inputs["aug_inter_feat"], f).reshape(B, CI * HW2)
    aorig = np.asarray(inputs["aug_origin_feat"], f).reshape(B, CO * HO2)
    w = _prep_weights(
        inputs["shallow_conv_w"], inputs["ofc_w1"], inputs["ofc_w2"],
        inputs["ofc_w3"], inputs["sfc_w1"], inputs["sfc_w2"], inputs["sfc_w3"],
        inputs["tfc_w1"], inputs["tfc_w2"], inputs["cfc_w1"], inputs["cfc_w2"],
        inputs["oc_w1"], inputs["oc_w2"], inputs["center"], inputs["protos"],
        inputs["epoch"],
    )
    in_maps = []
    for c in range(NCORE):
        sl = slice(c * BC, (c + 1) * BC)
        m = dict(w)
        m["xi"] = np.ascontiguousarray(inter[sl])
        m["xo"] = np.ascontiguousarray(orig[sl])
        m["ai"] = np.ascontiguousarray(ainter[sl])
        m["ao"] = np.ascontiguousarray(aorig[sl])
        in_maps.append(m)

    res = run_bass_kernel_spmd(nc, in_maps, core_ids=list(range(NCORE)),
                               trace=trace)
    rows = np.stack([res.results[c]["out"][0] for c in range(NCORE)])  # [8, 264]
    osv0 = rows[:, 0:64].astype(f)
    csv0 = rows[:, 64:128].astype(f)
    osv1 = rows[:, 128:192].astype(f)
    csv1 = rows[:, 192:256].astype(f)
    s01 = rows[:, 256:260].astype(f)
    s10 = rows[:, 260:264].astype(f)

    l01 = f(-(s01.sum(dtype=f)) / B)
    l10 = f(-(s10.sum(dtype=f)) / B)
    distill = f((l01 + l10) / 2.0)
    row_o = f(osv0.sum(dtype=f) / B + osv1.sum(dtype=f) / B)
    row_c = f(csv0.sum(dtype=f) / B + csv1.sum(dtype=f) / B)
    row_a = f(np.abs(osv0 - csv0).sum(dtype=f) / B
              + np.abs(osv1 - csv1).sum(dtype=f) / B)
    out = np.array([[distill], [row_o], [row_c], [row_a]], dtype=f)
    return out, res


def kernel(**inputs):
    out, _ = _run(inputs, trace=False)
    return out
